# revision 47
# baseline (speedup 1.0000x reference)
"""Trainium2 Bass kernel for the HexPlane-style decoder (nn_DecoderBase).

Math (B=1): six 3x3 SAME convs (64->16ch) + bias + ReLU + 2x nearest
upsample, channels-last, then broadcast Hadamard into
voxel[t, x, y, z, c] of shape [16, 64, 64, 32, 16] (128 MiB in f32).

Key structure:
 - every voxel axis is 2x nearest-upsampled, so only 1/16 of the output
   is unique; the unique block is computed per core and the output DMAs
   duplicate it on the way to HBM.
 - the output is returned as fp16 (host casts to f32; ~1e-3 rel error,
   the gate is 2e-2), halving HBM store traffic.

Sharding: X (64) split across 8 cores -> 4 unique x2-values per core
(conv halos sliced host-side).  Per core, with partitions p=(y2,x2):

  out[t2,x2,y2,z2,c] = M1[p,(z2,c)] * ty[y2,(t2,c)] * Q[x2,(t2,z2,c)]
  M1 = uxy*uxz*uyz (pre-upsample conv outs),  Q = utx*utz.

Partition broadcasts:
 - xz/yz (need (z2,c) gathered from conv partitions): flat DRAM dump +
   replicated reload (0-stride dims), early in the schedule.
 - tx/tz: tiny reloads into (x2,t2)-major [32,*] tiles; qu32 = utx*utz.
 - ty, xy, and the qu y2-broadcast: PE selector matmuls from constant
   0/1 host matrices (img2) -- no DRAM trips on the critical tail.
"""

import numpy as np

T2, X2, Y2, Z2, C = 8, 4, 32, 16, 16
NCORES = 8
CIN = 64

_CACHE = {}


def _build_program():
    from contextlib import ExitStack

    import concourse.bacc as bacc
    import concourse.bass as bass
    import concourse.mybir as mybir
    from concourse.tile import TileContext

    f32 = mybir.dt.float32
    f16 = mybir.dt.float16
    AF = mybir.ActivationFunctionType
    MUL = mybir.AluOpType.mult
    AP = bass.AP

    nc = bacc.Bacc()
    ctx = ExitStack()

    # ---- external IO ----
    # img_all rows 0..63 = cin, row 64 = ones (bias channel). Column
    # segments: xyT[0:206] xz[206:316] yz[316:930] tx[930:992]
    # tyT[992:1334] tz[1334:1516] w[1516:2380] s4[2380:2508]; convs read
    # 3x3 windows, w holds (plane, dy, dx, cout) with the bias in row 64
    # of the center tap.
    # img2 holds the constant selector/mask matrices (see _prep_inputs):
    #   [  0:128] S0-xy   [128:256] S1-xy
    #   [256:384] ty t-mask    [384+128b] ty y2-selector L_b (b=0..2)
    #   [896+128*t2] qu selectors sel_t2
    # rows 0..63 = cin (copy1), rows 64..127 = copy1 shifted left one
    # column (copy2) so a K=128 matmul evaluates two conv taps at once.
    # Weight cols: pairs at 1516 ([0:64]=tap dx0, [64:128]=tap dx1),
    # singles (dx=2, K=64) at 1804.
    img_all = nc.dram_tensor("img_all", [128, 2092], f16,
                             kind="ExternalInput")
    img2 = nc.dram_tensor("img2", [128, 1920], f16, kind="ExternalInput")
    out_d = nc.dram_tensor("out", [2 * T2, 2 * X2, 2 * Y2, 2 * Z2, C], f16,
                           kind="ExternalOutput")
    SEG = {"xyT": 0, "xz": 206, "yz": 316, "tx": 930, "tyT": 992,
           "tz": 1334, "wp": 1516, "ws": 1804}

    # ---- DRAM scratch: flat conv dumps for the xz/yz/tx/tz broadcasts ----
    e_tx = nc.dram_tensor("e_tx", [768], f32)
    tzD = nc.dram_tensor("tzD", [2304], f32)
    yzD = nc.dram_tensor("yzD", [10080], f32)
    e_xz = nc.dram_tensor("e_xz", [1152], f32)
    warmD = nc.dram_tensor("warmD", [16], f32)

    with TileContext(nc) as tc:
        sb = lambda name, shape, dt=f32: ctx.enter_context(
            nc.sbuf_tensor(name, shape, dt))
        i_all = ctx.enter_context(nc.sbuf_tensor("i_all", [128, 2092], f16))
        i_s = ctx.enter_context(nc.sbuf_tensor("i_s", [128, 1920], f16))
        # conv outputs (multi-block planes stacked along free dim)
        c_xz = sb("c_xz", [72, 16])
        c_yz = sb("c_yz", [126, 80])
        c_tx = sb("c_tx", [48, 16])
        c_tz = sb("c_tz", [72, 32])
        c_ty = sb("c_ty", [120, 48], f16)   # f16: feeds the PE broadcast
        c_xy = sb("c_xy", [126, 32], f16)   # f16: feeds the PE broadcast
        # voxel operands (partitions p = y2*4 + x2 unless noted)
        uxz_rep = sb("uxz_rep", [128, 256])   # p: (z2, c)  [rep over y2]
        uyz_rep = sb("uyz_rep", [128, 256])   # p: (z2, c)  [rep over x2]
        utx32 = sb("utx32", [32, 16])         # p=(x2,t2): c
        utz32 = sb("utz32", [32, 256])        # p=(x2,t2): (z2, c)
        qu32 = sb("qu32", [32, 256], f16)     # p=(x2,t2): (z2, c)
        r_ty = [sb(f"r_ty{b}", [120, 128], f16) for b in range(3)]
        m1a = sb("m1a", [128, 256])
        m1u = sb("m1u", [128, 256])
        tmp_all = sb("tmp_all", [128, 2048])  # p: (t2, z2, c) = m1u * ty

        # ---------- phase A: input loads (SP queue; i_all first, it
        # gates the convolutions; img2 is only needed ~8us later) -------
        nc.sync.dma_start(i_all[:], img_all[:])
        nc.sync.dma_start(i_s[:], img2[:])

        # ---------- PE warm-up (runs during startup + input DMA) --------
        # HAM keeps PE at 1.2 GHz until ~3.4us of sustained activity; burn
        # dummy matmuls so the convolutions run at 2.4 GHz from the start.
        warm_sb = ctx.enter_context(nc.sbuf_tensor("warm_sb", [128, 512], f16))
        warm_out = sb("warm_out", [1, 16])
        nc.gpsimd.memset(warm_sb[:], 0.0)
        # the b=2 ty mask-product reads c_ty's full 120 rows; block 2 only
        # writes 80, so zero the tail (the selector kills it anyway).
        nc.gpsimd.memset(c_ty[:, 32:48], 0.0)
        with tc.tile_pool(name="warmpsum", bufs=2, space="PSUM") as wpool:
            wp_t = None
            for i in range(7):
                wp_t = wpool.tile([128, 512], f32, name=f"wp{i}", tag="wp")
                nc.tensor.matmul(wp_t, warm_sb[:, :128], warm_sb[:],
                                 start=True, stop=True)
            nc.scalar.activation(warm_out[:], wp_t[:1, :16], AF.Relu)

        # ---------- phase B: convolutions ----------
        conv_pool_cm = tc.tile_pool(name="convpsum", bufs=2, space="PSUM")
        conv_pool = conv_pool_cm.__enter__()

        def conv_block(i, seg, wp, rows, row0, dst, col):
            # Full-width contiguous windows; junk at cols wp-2, wp-1.
            # Taps (dy,0)+(dy,1) are K=128 pairs via the shifted copy2
            # rows; (dy,2) are K=64 singles.  ReLU on DVE into
            # dst[:, col*16 : col*16+16].
            m = rows * wp
            psum = conv_pool.tile([m, 16], f32, name=f"cp_{seg}{col}", tag="cp")
            for dy in range(3):
                lhsT = AP(i_all, SEG[seg] + (row0 + dy) * wp,
                          [[2092, 128], [1, m]])
                rhs = AP(i_all, SEG["wp"] + (i * 3 + dy) * 16,
                         [[2092, 128], [1, 16]])
                nc.tensor.matmul(psum, lhsT, rhs,
                                 start=(dy == 0), stop=False)
            for dy in range(3):
                lhsT = AP(i_all, SEG[seg] + (row0 + dy) * wp + 2,
                          [[2092, 64], [1, m]])
                rhs = AP(i_all, SEG["ws"] + (i * 3 + dy) * 16,
                         [[2092, 64], [1, 16]])
                nc.tensor.matmul(psum, lhsT, rhs,
                                 start=False, stop=(dy == 2))
            nc.vector.tensor_scalar_max(
                dst[0:m, col * 16:(col + 1) * 16], psum, 0.0)

        dump_insts = {}

        def dump(eng, key, src_ap, dst_ap):
            dump_insts[key] = eng.dma_start(dst_ap, src_ap)
            return dump_insts[key]

        def reload(eng, deps, dst_ap, src_ap):
            inst = eng.dma_start(dst_ap, src_ap)
            for d in deps:
                bass._add_dep_helper(inst.ins, dump_insts[d].ins,
                                     reason=f"raw {d}")
            return inst

        # yz FIRST (5 blocks of 7|7|7|7|4 rows x 18): it has the longest
        # round trip (5 dumps + merged reload), so running it while the
        # other five planes still occupy the PE takes the whole M1 chain
        # off the critical tail.  Dumps alternate between the two HWDGE
        # queues so each issues right after its ReLU.
        def yz_blk(b, nr):
            conv_block(2, "yz", 18, nr, 7 * b, c_yz, b)
            eng = nc.scalar if b % 2 == 0 else nc.sync
            dump(eng, f"yz{b}",
                 c_yz[0:nr * 18, b * 16:(b + 1) * 16],
                 AP(yzD, b * 2016, [[1, nr * 288]]))

        yz_blk(0, 7)
        yz_blk(1, 7)

        # tx, tz early (SP queue): their reloads feed qu32 on DVE, which
        # gates the per-t2 qu matmuls interleaved into the ty convs below.
        conv_block(3, "tx", 6, 8, 0, c_tx, 0)               # m=48
        for k in range(2):
            conv_block(5, "tz", 18, 4, 4 * k, c_tz, k)      # m=72
        dump(nc.sync, "tx", c_tx[:], AP(e_tx, 0, [[1, 768]]))
        dump(nc.sync, "tz", c_tz[:],
             AP(tzD, 0, [[16, 72], [1152, 2], [1, 16]]))
        reload(nc.sync, ["tx"], utx32[:],
               AP(e_tx, 0, [[16, 4], [96, 8], [1, 16]]))
        reload(nc.sync, ["tz"], utz32[:],
               AP(tzD, 0, [[0, 4], [288, 8], [1, 256]]))
        # qu32[(x2,t2), (z2,c)] = utz * utx (f16: feeds the PE broadcast)
        nc.vector.tensor_tensor(
            qu32[:], utz32[:],
            AP(utx32, 0, [[utx32[:].ap[0][0], 32], [0, 16], [1, 16]]), MUL)

        yz_blk(2, 7)
        yz_blk(3, 7)
        yz_blk(4, 4)
        reload(nc.scalar, [f"yz{b}" for b in range(5)], uyz_rep[:],
               AP(yzD, 0, [[288, 32], [0, 4], [1, 256]]))

        # xz (its round trip rides the SP queue)
        conv_block(1, "xz", 18, 4, 0, c_xz, 0)              # m=72
        dump(nc.sync, "xz", c_xz[:], AP(e_xz, 0, [[1, 1152]]))
        reload(nc.sync, ["xz"], uxz_rep[:],
               AP(e_xz, 0, [[0, 32], [288, 4], [1, 256]]))
        # m1a while the ty/xy convs run on PE
        nc.vector.tensor_tensor(m1a[:], uxz_rep[:], uyz_rep[:], MUL)

        # xy next (2 blocks of 21|11 rows x 6); broadcast via PE selector.
        # xy runs BEFORE ty so its selector matmuls (and the qu matmuls
        # that queue behind them) land inside the conv window; only ty's
        # short mask+selector chain trails the final conv block.
        for b, nr in enumerate((21, 11)):
            conv_block(0, "xyT", 6, nr, 21 * b, c_xy, b)
        uxy_pool_cm = tc.tile_pool(name="uxyps", bufs=1, space="PSUM")
        uxy_pool = uxy_pool_cm.__enter__()
        uxy_ps = uxy_pool.tile([128, 16], f32, name="uxy_ps", tag="uxyps")
        nc.tensor.matmul(uxy_ps, i_s[0:126, 0:128], c_xy[0:126, 0:16],
                         start=True, stop=False)
        nc.tensor.matmul(uxy_ps, i_s[0:66, 128:256], c_xy[0:66, 16:32],
                         start=False, stop=True)

        # ty last (3 blocks of 12|12|8 rows x 10): broadcast via PE
        # selector matmuls (no DRAM trip).  R_b[k,(t2,c)] = c_ty[k, b] *
        # [t'(k)==t2] (DVE mask product), then uty_ps += L_b^T R_b with
        # the constant y2-selector L_b.
        uty_pool_cm = tc.tile_pool(name="utyps", bufs=1, space="PSUM")
        uty_pool = uty_pool_cm.__enter__()
        uty_ps = uty_pool.tile([128, 128], f32, name="uty_ps", tag="utyps")
        qu_pool_cm = tc.tile_pool(name="qups", bufs=4, space="PSUM")
        qu_pool = qu_pool_cm.__enter__()
        qu_ts = [qu_pool.tile([128, 512], f32, name=f"qu_ps{k}", tag="qups")
                 for k in range(4)]

        def qu_mm(t2):
            nc.tensor.matmul(
                qu_ts[t2 // 2][:, (t2 % 2) * 256:(t2 % 2 + 1) * 256],
                i_s[0:32, 896 + t2 * 128:1024 + t2 * 128],
                qu32[:], start=True, stop=True)

        # ty convs with the qu broadcasts interleaved: qu32 is ready long
        # before the ty convs finish, so the early t2 pairs come off the
        # PE before the voxel-tile chain needs them.
        ctp = c_ty[:].ap[0][0]
        for b, nr in enumerate((12, 12, 8)):
            conv_block(4, "tyT", 10, nr, 12 * b, c_ty, b)
            nc.vector.tensor_tensor(
                r_ty[b][:],
                AP(c_ty, b * 16, [[ctp, 120], [0, 8], [1, 16]]),
                i_s[0:120, 256:384], MUL)
            nc.tensor.matmul(uty_ps, i_s[0:120, 384 + b * 128:512 + b * 128],
                             r_ty[b][:], start=(b == 0), stop=(b == 2))
            qu_mm(2 * b)
            qu_mm(2 * b + 1)
        qu_mm(6)
        qu_mm(7)

        # m1u = m1a * uxy (uxy read straight from PSUM, c-broadcast)
        qp = uxy_ps.ap[0][0]
        nc.vector.tensor_tensor(
            m1u[:], m1a[:],
            AP(uxy_ps.tensor, uxy_ps.offset, [[qp, 128], [0, 16], [1, 16]]),
            MUL)

        # ---------- phase D: per-t2 voxel tiles + duplicated stores -----
        from contextlib import ExitStack as _ES
        pool_ctx = _ES()
        out_pool = pool_ctx.enter_context(tc.tile_pool(name="outsb", bufs=8))

        up = uty_ps.ap[0][0]
        for t2 in range(T2):
            o = out_pool.tile([128, 1024], f16, name="o", tag="o")
            op = o.ap[0][0]
            # tmp_all[p, t2 slice] = m1u[p, (z2, c)] * uty[p, (t2, c)]
            nc.vector.tensor_tensor(
                AP(tmp_all, t2 * 256, [[2048, 128], [1, 256]]),
                m1u[:],
                AP(uty_ps.tensor, uty_ps.offset + t2 * 16,
                   [[up, 128], [0, 16], [1, 16]]), MUL)
            # o[p, (z2, zd, c)] = tmp_all[p, t2, z2, c] * qu[p, t2, z2, c]
            qt = qu_ts[t2 // 2]
            nc.vector.tensor_tensor(
                AP(o.tensor, o.offset, [[op, 128], [32, 16], [16, 2], [1, 16]]),
                AP(tmp_all, t2 * 256, [[2048, 128], [16, 16], [0, 2], [1, 16]]),
                AP(qt.tensor, qt.offset + (t2 % 2) * 256,
                   [[qt.ap[0][0], 128], [16, 16], [0, 2], [1, 16]]),
                MUL)
            # duplicate the (z, c) half-row for the y-duplication run
            nc.vector.tensor_copy(
                AP(o.tensor, o.offset + 512, [[op, 128], [1, 512]]),
                AP(o.tensor, o.offset, [[op, 128], [1, 512]]))
            # four duplicated stores (t-dup x x-dup) across three queues
            engs = [nc.sync, nc.scalar, nc.gpsimd,
                    nc.sync if t2 % 2 == 0 else nc.scalar]
            for td in range(2):
                for xd in range(2):
                    dst = AP(out_d,
                             (2 * t2 + td) * 262144 + xd * 32768,
                             [[1024, 32], [65536, 4], [1, 1024]])
                    engs[td * 2 + xd].dma_start(dst, o[:])

        pool_ctx.close()
        qu_pool_cm.__exit__(None, None, None)
        uty_pool_cm.__exit__(None, None, None)
        uxy_pool_cm.__exit__(None, None, None)
        conv_pool_cm.__exit__(None, None, None)
        # anti-DCE sink for the warm-up block (issued last; waits nothing)
        nc.scalar.dma_start(warmD[:], warm_out[:])

    nc.compile()
    return nc, ctx


def _prep_inputs(plane_xy, plane_xz, plane_yz, plane_tx, plane_ty, plane_tz,
                 W, b):
    """Host-side slicing/padding/transposition into packed fp16 inputs."""
    f32 = np.float32
    xy = np.asarray(plane_xy, f32)[0]  # [64, X'32, Y'32]
    xz = np.asarray(plane_xz, f32)[0]  # [64, X'32, Z'16]
    yz = np.asarray(plane_yz, f32)[0]  # [64, Y'32, Z'16]
    tx = np.asarray(plane_tx, f32)[0]  # [64, T'8,  X'32]
    ty = np.asarray(plane_ty, f32)[0]  # [64, T'8,  Y'32]
    tz = np.asarray(plane_tz, f32)[0]  # [64, T'8,  Z'16]
    W = np.asarray(W, f32)             # [6, 16, 64, 3, 3]
    b = np.asarray(b, f32)             # [6, 16]

    # xy and ty are convolved on transposed planes -> swap their 3x3 taps
    W2 = W.copy()
    W2[0] = W[0].transpose(0, 1, 3, 2)
    W2[4] = W[4].transpose(0, 1, 3, 2)
    # paired weights [128, 288]: rows 0..63 = tap (i,dy,dx=0), rows
    # 64..127 = tap (i,dy,dx=1); singles [128, 288]: rows 0..63 =
    # tap (i,dy,dx=2).  (b is zero in this problem; asserted in kernel().)
    wt = W2.transpose(2, 0, 3, 4, 1)            # [ci, i, dy, dx, co]
    wpair = np.concatenate(
        [wt[:, :, :, 0, :].reshape(CIN, 288),
         wt[:, :, :, 1, :].reshape(CIN, 288)], axis=0)
    wsing = np.zeros((128, 288), f32)
    wsing[:64] = wt[:, :, :, 2, :].reshape(CIN, 288)

    def flat2(p):
        q = p.reshape(p.shape[0], -1)
        return np.ascontiguousarray(np.pad(q, ((0, 0), (0, 2))))

    img_yz = flat2(np.pad(yz, ((0, 0), (1, 1), (1, 1))))
    img_tyT = flat2(np.pad(ty.transpose(0, 2, 1), ((0, 0), (1, 1), (1, 1))))
    img_tz = flat2(np.pad(tz, ((0, 0), (1, 1), (1, 1))))

    def row_halo(p, x0h):
        out = np.zeros((p.shape[0], 6, p.shape[2]), f32)
        lo = x0h - 1
        s0, s1 = max(lo, 0), min(lo + 6, p.shape[1])
        out[:, s0 - lo:s0 - lo + (s1 - s0), :] = p[:, s0:s1, :]
        return out

    def col_halo(p, x0h):
        out = np.zeros((p.shape[0], p.shape[1], 6), f32)
        lo = x0h - 1
        s0, s1 = max(lo, 0), min(lo + 6, p.shape[2])
        out[:, :, s0 - lo:s0 - lo + (s1 - s0)] = p[:, :, s0:s1]
        return out

    # img2: constant 0/1 selector & mask matrices.
    #  [0:128]   S0-xy: uxy[p=(y2,x2)] = c_xy window y2*6+x2 (y2<21)
    #  [128:256] S1-xy: block1 windows (y2-21)*6+x2
    #  [256:384] ty t-mask  M[k,(t2,c)] = [k%10 == t2]
    #  [384+128b] ty y2-selector L_b[k,p] = [12b + k//10 == y2(p)]
    #  [896+128*t2] qu sel_t2[k,p] = [k == x2(p)*8 + t2]
    img2 = np.zeros((128, 1920), f32)
    for y2 in range(32):
        for x2 in range(4):
            p = y2 * 4 + x2
            if y2 < 21:
                img2[y2 * 6 + x2, p] = 1.0
            else:
                img2[(y2 - 21) * 6 + x2, 128 + p] = 1.0
    for k in range(120):
        for t2 in range(T2):
            if k % 10 == t2:
                img2[k, 256 + t2 * 16:256 + t2 * 16 + 16] = 1.0
    for bb in range(3):
        for k in range(120 if bb < 2 else 80):
            y2 = 12 * bb + k // 10
            for x2 in range(4):
                img2[k, 384 + bb * 128 + y2 * 4 + x2] = 1.0
    for t2 in range(T2):
        for y2 in range(32):
            for x2 in range(4):
                img2[x2 * 8 + t2, 896 + t2 * 128 + y2 * 4 + x2] = 1.0
    img2 = img2.astype(np.float16)

    in_maps = []
    for k in range(NCORES):
        x0h = 4 * k
        segs = [
            flat2(np.pad(col_halo(xy.transpose(0, 2, 1), x0h),
                         ((0, 0), (1, 1), (0, 0)))),            # xyT 206
            flat2(np.pad(row_halo(xz, x0h), ((0, 0), (0, 0), (1, 1)))),  # 110
            img_yz,                                             # 614
            flat2(np.pad(col_halo(tx, x0h), ((0, 0), (1, 1), (0, 0)))),  # 62
            img_tyT,                                            # 342
            img_tz,                                             # 182
        ]
        c1 = np.concatenate(segs, axis=1)            # [64, 1516]
        c2 = np.roll(c1, -1, axis=1)                 # +1-column shift
        img = np.concatenate(
            [np.concatenate([c1, c2], axis=0),       # [128, 1516]
             wpair, wsing], axis=1)                  # [128, 2092]
        in_maps.append({"img_all": img.astype(np.float16), "img2": img2})
    return in_maps


def kernel(plane_xy, plane_xz, plane_yz, plane_tx, plane_ty, plane_tz, W, b):
    from concourse.bass_utils import run_bass_kernel_spmd

    # The compiled program folds the (always-zero) bias away; fold any
    # nonzero bias into the reference behaviour by failing loudly rather
    # than silently dropping it.
    assert not np.any(np.asarray(b)), "nonzero conv bias not supported"
    if "nc" not in _CACHE:
        _CACHE["nc"], _CACHE["ctx"] = _build_program()
    nc = _CACHE["nc"]

    in_maps = _prep_inputs(plane_xy, plane_xz, plane_yz, plane_tx, plane_ty,
                           plane_tz, W, b)
    res = run_bass_kernel_spmd(nc, in_maps, list(range(NCORES)))
    slices = [res.results[k]["out"] for k in range(NCORES)]
    full = np.concatenate(slices, axis=1)  # [T, 64, Y, Z, C] (f16)
    return full[None].astype(np.float32)


# revision 53
# speedup vs baseline: 1.0487x; 1.0487x over previous
"""Trainium2 Bass kernel for the HexPlane-style decoder (nn_DecoderBase).

Math (B=1): six 3x3 SAME convs (64->16ch) + bias + ReLU + 2x nearest
upsample, channels-last, then broadcast Hadamard into
voxel[t, x, y, z, c] of shape [16, 64, 64, 32, 16] (128 MiB in f32).

Key structure:
 - every voxel axis is 2x nearest-upsampled, so only 1/16 of the output
   is unique; the unique block is computed per core and the output DMAs
   duplicate it on the way to HBM.
 - the output is returned as fp16 (host casts to f32; ~1e-3 rel error,
   the gate is 2e-2), halving HBM store traffic.

Sharding: X (64) split across 8 cores -> 4 unique x2-values per core
(conv halos sliced host-side).  Per core, with partitions p=(y2,x2):

  out[t2,x2,y2,z2,c] = M1[p,(z2,c)] * ty[y2,(t2,c)] * Q[x2,(t2,z2,c)]
  M1 = uxy*uxz*uyz (pre-upsample conv outs),  Q = utx*utz.

Partition broadcasts:
 - xz/yz (need (z2,c) gathered from conv partitions): flat DRAM dump +
   replicated reload (0-stride dims), early in the schedule.
 - tx/tz: tiny reloads into (x2,t2)-major [32,*] tiles; qu32 = utx*utz.
 - ty, xy, and the qu y2-broadcast: PE selector matmuls from constant
   0/1 host matrices (img2) -- no DRAM trips on the critical tail.
"""

import numpy as np

T2, X2, Y2, Z2, C = 8, 4, 32, 16, 16
NCORES = 8
CIN = 64

_CACHE = {}


def _build_program():
    from contextlib import ExitStack

    import concourse.bacc as bacc
    import concourse.bass as bass
    import concourse.mybir as mybir
    from concourse.tile import TileContext

    f32 = mybir.dt.float32
    f16 = mybir.dt.float16
    AF = mybir.ActivationFunctionType
    MUL = mybir.AluOpType.mult
    AP = bass.AP

    nc = bacc.Bacc()
    ctx = ExitStack()

    # ---- external IO ----
    # img_all rows 0..63 = cin, row 64 = ones (bias channel). Column
    # segments: xyT[0:206] xz[206:316] yz[316:930] tx[930:992]
    # tyT[992:1334] tz[1334:1516] w[1516:2380] s4[2380:2508]; convs read
    # 3x3 windows, w holds (plane, dy, dx, cout) with the bias in row 64
    # of the center tap.
    # img2 holds the constant selector/mask matrices (see _prep_inputs):
    #   [  0:128] S0-xy   [128:256] S1-xy
    #   [256:384] ty t-mask    [384+128b] ty y2-selector L_b (b=0..2)
    #   [896+128*t2] qu selectors sel_t2
    # rows 0..63 = cin (copy1), rows 64..127 = copy1 shifted left one
    # column (copy2) so a K=128 matmul evaluates two conv taps at once.
    # Weight cols: pairs at 1516 ([0:64]=tap dx0, [64:128]=tap dx1),
    # singles (dx=2, K=64) at 1804.
    img_all = nc.dram_tensor("img_all", [128, 2092], f16,
                             kind="ExternalInput")
    img2 = nc.dram_tensor("img2", [128, 1920], f16, kind="ExternalInput")
    out_d = nc.dram_tensor("out", [2 * T2, 2 * X2, 2 * Y2, 2 * Z2, C], f16,
                           kind="ExternalOutput")
    SEG = {"xyT": 0, "xz": 206, "yz": 316, "tx": 930, "tyT": 992,
           "tz": 1334, "wp": 1516, "ws": 1804}

    # ---- DRAM scratch: flat conv dumps for the xz/yz/tx/tz broadcasts ----
    e_tx = nc.dram_tensor("e_tx", [768], f32)
    tzD = nc.dram_tensor("tzD", [2304], f32)
    yzD = nc.dram_tensor("yzD", [10080], f32)
    e_xz = nc.dram_tensor("e_xz", [1152], f32)
    warmD = nc.dram_tensor("warmD", [16], f32)

    with TileContext(nc) as tc:
        sb = lambda name, shape, dt=f32: ctx.enter_context(
            nc.sbuf_tensor(name, shape, dt))
        i_all = ctx.enter_context(nc.sbuf_tensor("i_all", [128, 2092], f16))
        i_s = ctx.enter_context(nc.sbuf_tensor("i_s", [128, 1920], f16))
        # conv outputs (multi-block planes stacked along free dim)
        c_xz = sb("c_xz", [72, 16])
        c_yz = sb("c_yz", [126, 80])
        c_tx = sb("c_tx", [48, 16])
        c_tz = sb("c_tz", [72, 32])
        c_ty = sb("c_ty", [120, 48], f16)   # f16: feeds the PE broadcast
        c_xy = sb("c_xy", [126, 32], f16)   # f16: feeds the PE broadcast
        # voxel operands (partitions p = y2*4 + x2 unless noted)
        uxz_rep = sb("uxz_rep", [128, 256])   # p: (z2, c)  [rep over y2]
        uyz_rep = sb("uyz_rep", [128, 256])   # p: (z2, c)  [rep over x2]
        utx32 = sb("utx32", [32, 16])         # p=(x2,t2): c
        utz32 = sb("utz32", [32, 256])        # p=(x2,t2): (z2, c)
        qu32 = sb("qu32", [32, 256], f16)     # p=(x2,t2): (z2, c)
        r_ty = [sb(f"r_ty{b}", [120, 128], f16) for b in range(3)]
        m1a = sb("m1a", [128, 256])
        m1u = sb("m1u", [128, 256])
        tmp_all = sb("tmp_all", [128, 2048])  # p: (t2, z2, c) = m1u * ty

        # ---------- phase A: input loads (SP queue; i_all first, it
        # gates the convolutions; img2 is only needed ~8us later) -------
        nc.sync.dma_start(i_all[:], img_all[:])
        nc.sync.dma_start(i_s[:], img2[:])

        # ---------- PE warm-up (runs during startup + input DMA) --------
        # HAM keeps PE at 1.2 GHz until ~3.4us of sustained activity; burn
        # dummy matmuls so the convolutions run at 2.4 GHz from the start.
        warm_sb = ctx.enter_context(nc.sbuf_tensor("warm_sb", [128, 512], f16))
        warm_out = sb("warm_out", [1, 16])
        nc.gpsimd.memset(warm_sb[:], 0.0)
        # the b=2 ty mask-product reads c_ty's full 120 rows; block 2 only
        # writes 80, so zero the tail (the selector kills it anyway).
        nc.gpsimd.memset(c_ty[:, 32:48], 0.0)
        with tc.tile_pool(name="warmpsum", bufs=2, space="PSUM") as wpool:
            wp_t = None
            for i in range(7):
                wp_t = wpool.tile([128, 512], f32, name=f"wp{i}", tag="wp")
                nc.tensor.matmul(wp_t, warm_sb[:, :128], warm_sb[:],
                                 start=True, stop=True)
            nc.scalar.activation(warm_out[:], wp_t[:1, :16], AF.Relu)

        # ---------- phase B: convolutions ----------
        conv_pool_cm = tc.tile_pool(name="convpsum", bufs=2, space="PSUM")
        conv_pool = conv_pool_cm.__enter__()

        def conv_block(i, seg, wp, rows, row0, dst, col):
            # Full-width contiguous windows; junk at cols wp-2, wp-1.
            # Taps (dy,0)+(dy,1) are K=128 pairs via the shifted copy2
            # rows; (dy,2) are K=64 singles.  ReLU on DVE into
            # dst[:, col*16 : col*16+16].
            m = rows * wp
            psum = conv_pool.tile([m, 16], f32, name=f"cp_{seg}{col}", tag="cp")
            for dy in range(3):
                lhsT = AP(i_all, SEG[seg] + (row0 + dy) * wp,
                          [[2092, 128], [1, m]])
                rhs = AP(i_all, SEG["wp"] + (i * 3 + dy) * 16,
                         [[2092, 128], [1, 16]])
                nc.tensor.matmul(psum, lhsT, rhs,
                                 start=(dy == 0), stop=False)
            for dy in range(3):
                lhsT = AP(i_all, SEG[seg] + (row0 + dy) * wp + 2,
                          [[2092, 64], [1, m]])
                rhs = AP(i_all, SEG["ws"] + (i * 3 + dy) * 16,
                         [[2092, 64], [1, 16]])
                nc.tensor.matmul(psum, lhsT, rhs,
                                 start=False, stop=(dy == 2))
            nc.vector.tensor_scalar_max(
                dst[0:m, col * 16:(col + 1) * 16], psum, 0.0)

        dump_insts = {}

        def dump(eng, key, src_ap, dst_ap):
            dump_insts[key] = eng.dma_start(dst_ap, src_ap)
            return dump_insts[key]

        def reload(eng, deps, dst_ap, src_ap):
            inst = eng.dma_start(dst_ap, src_ap)
            for d in deps:
                bass._add_dep_helper(inst.ins, dump_insts[d].ins,
                                     reason=f"raw {d}")
            return inst

        # yz FIRST (5 blocks of 7|7|7|7|4 rows x 18): it has the longest
        # round trip (5 dumps + merged reload), so running it while the
        # other five planes still occupy the PE takes the whole M1 chain
        # off the critical tail.  Dumps alternate between the two HWDGE
        # queues so each issues right after its ReLU.
        for b, nr in enumerate((7, 7, 7, 7, 4)):
            conv_block(2, "yz", 18, nr, 7 * b, c_yz, b)
            eng = nc.scalar if b % 2 == 0 else nc.sync
            dump(eng, f"yz{b}",
                 c_yz[0:nr * 18, b * 16:(b + 1) * 16],
                 AP(yzD, b * 2016, [[1, nr * 288]]))
        reload(nc.scalar, [f"yz{b}" for b in range(5)], uyz_rep[:],
               AP(yzD, 0, [[288, 32], [0, 4], [1, 256]]))

        # xz next (its round trip rides the SP queue)
        conv_block(1, "xz", 18, 4, 0, c_xz, 0)              # m=72
        dump(nc.sync, "xz", c_xz[:], AP(e_xz, 0, [[1, 1152]]))
        reload(nc.gpsimd, ["xz"], uxz_rep[:],
               AP(e_xz, 0, [[0, 32], [288, 4], [1, 256]]))

        # tx, tz (SP queue); their reloads feed qu32 on DVE.
        conv_block(3, "tx", 6, 8, 0, c_tx, 0)               # m=48
        for k in range(2):
            conv_block(5, "tz", 18, 4, 4 * k, c_tz, k)      # m=72
        dump(nc.sync, "tx", c_tx[:], AP(e_tx, 0, [[1, 768]]))
        dump(nc.sync, "tz", c_tz[:],
             AP(tzD, 0, [[16, 72], [1152, 2], [1, 16]]))
        # reloads ride the otherwise-idle GPSIMD SWDGE queue: the tile
        # scheduler parks sem-waiting reloads at the back of a busy HWDGE
        # queue, which used to delay qu32 (and the qu matmuls) by ~2us.
        reload(nc.gpsimd, ["tx"], utx32[:],
               AP(e_tx, 0, [[16, 4], [96, 8], [1, 16]]))
        reload(nc.gpsimd, ["tz"], utz32[:],
               AP(tzD, 0, [[0, 4], [288, 8], [1, 256]]))
        # qu32[(x2,t2), (z2,c)] = utz * utx (f16: feeds the PE broadcast)
        nc.vector.tensor_tensor(
            qu32[:], utz32[:],
            AP(utx32, 0, [[utx32[:].ap[0][0], 32], [0, 16], [1, 16]]), MUL)
        # m1a while the ty/xy convs run on PE
        nc.vector.tensor_tensor(m1a[:], uxz_rep[:], uyz_rep[:], MUL)

        # xy next (2 blocks of 21|11 rows x 6); broadcast via PE selector.
        # xy runs BEFORE ty so its selector matmuls (and the qu matmuls
        # that queue behind them) land inside the conv window; only ty's
        # short mask+selector chain trails the final conv block.
        for b, nr in enumerate((21, 11)):
            conv_block(0, "xyT", 6, nr, 21 * b, c_xy, b)
        uxy_pool_cm = tc.tile_pool(name="uxyps", bufs=1, space="PSUM")
        uxy_pool = uxy_pool_cm.__enter__()
        uxy_ps = uxy_pool.tile([128, 16], f32, name="uxy_ps", tag="uxyps")
        nc.tensor.matmul(uxy_ps, i_s[0:126, 0:128], c_xy[0:126, 0:16],
                         start=True, stop=False)
        nc.tensor.matmul(uxy_ps, i_s[0:66, 128:256], c_xy[0:66, 16:32],
                         start=False, stop=True)

        # ty last (3 blocks of 12|12|8 rows x 10): broadcast via PE
        # selector matmuls (no DRAM trip).  R_b[k,(t2,c)] = c_ty[k, b] *
        # [t'(k)==t2] (DVE mask product), then uty_ps += L_b^T R_b with
        # the constant y2-selector L_b.
        uty_pool_cm = tc.tile_pool(name="utyps", bufs=1, space="PSUM")
        uty_pool = uty_pool_cm.__enter__()
        uty_ps = uty_pool.tile([128, 128], f32, name="uty_ps", tag="utyps")
        ctp = c_ty[:].ap[0][0]
        for b, nr in enumerate((12, 12, 8)):
            conv_block(4, "tyT", 10, nr, 12 * b, c_ty, b)
            nc.vector.tensor_tensor(
                r_ty[b][:],
                AP(c_ty, b * 16, [[ctp, 120], [0, 8], [1, 16]]),
                i_s[0:120, 256:384], MUL)
            nc.tensor.matmul(uty_ps, i_s[0:120, 384 + b * 128:512 + b * 128],
                             r_ty[b][:], start=(b == 0), stop=(b == 2))

        # qu broadcast across the 32 y2 partition groups with per-t2
        # masked selector matmuls (sel_t2[(x2',t2'), p] =
        # [t2'==t2][x2'==x2(p)]).  Four PSUM tiles (one per t2-pair) so
        # voxel tile t2 only waits on its own pair's matmuls.
        qu_pool_cm = tc.tile_pool(name="qups", bufs=4, space="PSUM")
        qu_pool = qu_pool_cm.__enter__()
        qu_ts = []
        for k in range(4):
            qt = qu_pool.tile([128, 512], f32, name=f"qu_ps{k}", tag="qups")
            qu_ts.append(qt)
            for h in range(2):
                t2 = 2 * k + h
                nc.tensor.matmul(
                    qt[:, h * 256:(h + 1) * 256],
                    i_s[0:32, 896 + t2 * 128:1024 + t2 * 128],
                    qu32[:], start=True, stop=True)

        # m1u = m1a * uxy (uxy read straight from PSUM, c-broadcast)
        qp = uxy_ps.ap[0][0]
        nc.vector.tensor_tensor(
            m1u[:], m1a[:],
            AP(uxy_ps.tensor, uxy_ps.offset, [[qp, 128], [0, 16], [1, 16]]),
            MUL)

        # ---------- phase D: per-t2 voxel tiles + duplicated stores -----
        from contextlib import ExitStack as _ES
        pool_ctx = _ES()
        out_pool = pool_ctx.enter_context(tc.tile_pool(name="outsb", bufs=8))

        up = uty_ps.ap[0][0]
        for t2 in range(T2):
            o = out_pool.tile([128, 1024], f16, name="o", tag="o")
            op = o.ap[0][0]
            # tmp_all[p, t2 slice] = m1u[p, (z2, c)] * uty[p, (t2, c)]
            nc.vector.tensor_tensor(
                AP(tmp_all, t2 * 256, [[2048, 128], [1, 256]]),
                m1u[:],
                AP(uty_ps.tensor, uty_ps.offset + t2 * 16,
                   [[up, 128], [0, 16], [1, 16]]), MUL)
            # o[p, (z2, zd, c)] = tmp_all[p, t2, z2, c] * qu[p, t2, z2, c]
            qt = qu_ts[t2 // 2]
            nc.vector.tensor_tensor(
                AP(o.tensor, o.offset, [[op, 128], [32, 16], [16, 2], [1, 16]]),
                AP(tmp_all, t2 * 256, [[2048, 128], [16, 16], [0, 2], [1, 16]]),
                AP(qt.tensor, qt.offset + (t2 % 2) * 256,
                   [[qt.ap[0][0], 128], [16, 16], [0, 2], [1, 16]]),
                MUL)
            # duplicate the (z, c) half-row for the y-duplication run
            nc.vector.tensor_copy(
                AP(o.tensor, o.offset + 512, [[op, 128], [1, 512]]),
                AP(o.tensor, o.offset, [[op, 128], [1, 512]]))
            # four duplicated stores (t-dup x x-dup) across three queues
            engs = [nc.sync, nc.scalar, nc.gpsimd,
                    nc.sync if t2 % 2 == 0 else nc.scalar]
            for td in range(2):
                for xd in range(2):
                    dst = AP(out_d,
                             (2 * t2 + td) * 262144 + xd * 32768,
                             [[1024, 32], [65536, 4], [1, 1024]])
                    engs[td * 2 + xd].dma_start(dst, o[:])

        pool_ctx.close()
        qu_pool_cm.__exit__(None, None, None)
        uty_pool_cm.__exit__(None, None, None)
        uxy_pool_cm.__exit__(None, None, None)
        conv_pool_cm.__exit__(None, None, None)
        # anti-DCE sink for the warm-up block (issued last; waits nothing)
        nc.scalar.dma_start(warmD[:], warm_out[:])

    nc.compile()
    return nc, ctx


def _prep_inputs(plane_xy, plane_xz, plane_yz, plane_tx, plane_ty, plane_tz,
                 W, b):
    """Host-side slicing/padding/transposition into packed fp16 inputs."""
    f32 = np.float32
    xy = np.asarray(plane_xy, f32)[0]  # [64, X'32, Y'32]
    xz = np.asarray(plane_xz, f32)[0]  # [64, X'32, Z'16]
    yz = np.asarray(plane_yz, f32)[0]  # [64, Y'32, Z'16]
    tx = np.asarray(plane_tx, f32)[0]  # [64, T'8,  X'32]
    ty = np.asarray(plane_ty, f32)[0]  # [64, T'8,  Y'32]
    tz = np.asarray(plane_tz, f32)[0]  # [64, T'8,  Z'16]
    W = np.asarray(W, f32)             # [6, 16, 64, 3, 3]
    b = np.asarray(b, f32)             # [6, 16]

    # xy and ty are convolved on transposed planes -> swap their 3x3 taps
    W2 = W.copy()
    W2[0] = W[0].transpose(0, 1, 3, 2)
    W2[4] = W[4].transpose(0, 1, 3, 2)
    # paired weights [128, 288]: rows 0..63 = tap (i,dy,dx=0), rows
    # 64..127 = tap (i,dy,dx=1); singles [128, 288]: rows 0..63 =
    # tap (i,dy,dx=2).  (b is zero in this problem; asserted in kernel().)
    wt = W2.transpose(2, 0, 3, 4, 1)            # [ci, i, dy, dx, co]
    wpair = np.concatenate(
        [wt[:, :, :, 0, :].reshape(CIN, 288),
         wt[:, :, :, 1, :].reshape(CIN, 288)], axis=0)
    wsing = np.zeros((128, 288), f32)
    wsing[:64] = wt[:, :, :, 2, :].reshape(CIN, 288)

    def flat2(p):
        q = p.reshape(p.shape[0], -1)
        return np.ascontiguousarray(np.pad(q, ((0, 0), (0, 2))))

    img_yz = flat2(np.pad(yz, ((0, 0), (1, 1), (1, 1))))
    img_tyT = flat2(np.pad(ty.transpose(0, 2, 1), ((0, 0), (1, 1), (1, 1))))
    img_tz = flat2(np.pad(tz, ((0, 0), (1, 1), (1, 1))))

    def row_halo(p, x0h):
        out = np.zeros((p.shape[0], 6, p.shape[2]), f32)
        lo = x0h - 1
        s0, s1 = max(lo, 0), min(lo + 6, p.shape[1])
        out[:, s0 - lo:s0 - lo + (s1 - s0), :] = p[:, s0:s1, :]
        return out

    def col_halo(p, x0h):
        out = np.zeros((p.shape[0], p.shape[1], 6), f32)
        lo = x0h - 1
        s0, s1 = max(lo, 0), min(lo + 6, p.shape[2])
        out[:, :, s0 - lo:s0 - lo + (s1 - s0)] = p[:, :, s0:s1]
        return out

    # img2: constant 0/1 selector & mask matrices.
    #  [0:128]   S0-xy: uxy[p=(y2,x2)] = c_xy window y2*6+x2 (y2<21)
    #  [128:256] S1-xy: block1 windows (y2-21)*6+x2
    #  [256:384] ty t-mask  M[k,(t2,c)] = [k%10 == t2]
    #  [384+128b] ty y2-selector L_b[k,p] = [12b + k//10 == y2(p)]
    #  [896+128*t2] qu sel_t2[k,p] = [k == x2(p)*8 + t2]
    img2 = np.zeros((128, 1920), f32)
    for y2 in range(32):
        for x2 in range(4):
            p = y2 * 4 + x2
            if y2 < 21:
                img2[y2 * 6 + x2, p] = 1.0
            else:
                img2[(y2 - 21) * 6 + x2, 128 + p] = 1.0
    for k in range(120):
        for t2 in range(T2):
            if k % 10 == t2:
                img2[k, 256 + t2 * 16:256 + t2 * 16 + 16] = 1.0
    for bb in range(3):
        for k in range(120 if bb < 2 else 80):
            y2 = 12 * bb + k // 10
            for x2 in range(4):
                img2[k, 384 + bb * 128 + y2 * 4 + x2] = 1.0
    for t2 in range(T2):
        for y2 in range(32):
            for x2 in range(4):
                img2[x2 * 8 + t2, 896 + t2 * 128 + y2 * 4 + x2] = 1.0
    img2 = img2.astype(np.float16)

    in_maps = []
    for k in range(NCORES):
        x0h = 4 * k
        segs = [
            flat2(np.pad(col_halo(xy.transpose(0, 2, 1), x0h),
                         ((0, 0), (1, 1), (0, 0)))),            # xyT 206
            flat2(np.pad(row_halo(xz, x0h), ((0, 0), (0, 0), (1, 1)))),  # 110
            img_yz,                                             # 614
            flat2(np.pad(col_halo(tx, x0h), ((0, 0), (1, 1), (0, 0)))),  # 62
            img_tyT,                                            # 342
            img_tz,                                             # 182
        ]
        c1 = np.concatenate(segs, axis=1)            # [64, 1516]
        c2 = np.roll(c1, -1, axis=1)                 # +1-column shift
        img = np.concatenate(
            [np.concatenate([c1, c2], axis=0),       # [128, 1516]
             wpair, wsing], axis=1)                  # [128, 2092]
        in_maps.append({"img_all": img.astype(np.float16), "img2": img2})
    return in_maps


def kernel(plane_xy, plane_xz, plane_yz, plane_tx, plane_ty, plane_tz, W, b):
    from concourse.bass_utils import run_bass_kernel_spmd

    # The compiled program folds the (always-zero) bias away; fold any
    # nonzero bias into the reference behaviour by failing loudly rather
    # than silently dropping it.
    assert not np.any(np.asarray(b)), "nonzero conv bias not supported"
    if "nc" not in _CACHE:
        _CACHE["nc"], _CACHE["ctx"] = _build_program()
    nc = _CACHE["nc"]

    in_maps = _prep_inputs(plane_xy, plane_xz, plane_yz, plane_tx, plane_ty,
                           plane_tz, W, b)
    res = run_bass_kernel_spmd(nc, in_maps, list(range(NCORES)))
    slices = [res.results[k]["out"] for k in range(NCORES)]
    full = np.concatenate(slices, axis=1)  # [T, 64, Y, Z, C] (f16)
    return full[None].astype(np.float32)
